# revision 14
# baseline (speedup 1.0000x reference)
"""Trainium2 Bass kernel for nn_BatchBeliefStep (batched EKF step).

Exploits structure of the reference: the Jacobian A has zero rows/cols 3,4,
so P_new is block-diagonal (3x3 block + two diagonal entries), K has only 2
nonzero entries per row, and the innovation covariance S is diagonal.

Per row only 13 input floats and 15 output floats matter; the host packs
these into dense records, the device runs a pure elementwise kernel over
them (data-parallel across 8 cores), and the host scatters the results back
into the full-shape outputs.

The outputs are split into two DRAM tensors by producing engine (DVE vs
ACT) because this compiler's DMA instruction encoding only affords one
sem-wait: each output tile must have a single writer engine.
"""

import math

import numpy as np

N_TOTAL = 500_000
N_CORES = 8
TILE_C = 245          # records per partition per tile
TILE_T = 2            # tiles per core
ROWS_PER_CORE = 128 * TILE_C * TILE_T  # 62720 (62500 real + 220 pad)

N_IN = 13             # packed input floats per row
N_OD = 9              # DVE-written output floats per row
N_OA = 7              # ACT-written output floats per row

_PI = float(np.float32(math.pi))


def _params_from_inputs(n1, n2_sigma, n2_kappa, gains, obs_gains):
    f = lambda v: float(np.float32(v))
    q = [f(np.exp(np.float32(2.0) * np.float32(v))) for v in np.asarray(n1)]
    e2s = [f(np.exp(np.float32(2.0) * np.float32(v))) for v in np.asarray(n2_sigma)]
    ekh = [f(np.exp(np.float32(v))) for v in np.asarray(n2_kappa)]
    g = [f(v) for v in np.asarray(gains)]
    og = [f(v) for v in np.asarray(obs_gains)]
    return dict(q=q, e2s=e2s, ekh=ekh, g=g, og=og, dt=0.1, eps=1e-6)


def build_program(params, C=TILE_C, T=TILE_T):
    """Build the Bass program.

    inp   (128*C*T, 13): [x2, a0, a1, x0, x1, p00, p11, p02, p12, p22,
                          p01, y0, y1]
    out_d (128*C*T, 9):  [angc*, ang, px, py, xn3, xn4, B01, B02, B12]
    out_a (128*C*T, 7):  [k3, k4, B00, B11, B22, pn33, pn44]
    (* = dummy; cos-angle used only as Sin input)
    """
    import concourse.bass as bass
    import concourse.mybir as mybir
    from concourse.tile import TileContext

    F32 = mybir.dt.float32
    OP = mybir.AluOpType
    ACTF = mybir.ActivationFunctionType

    dt = params["dt"]
    eps = params["eps"]
    g0, g1 = params["g"]
    og0, og1 = params["og"]
    q0, q1, q2, q3, q4 = params["q"]
    e2s0, e2s1 = params["e2s"]
    ekh0, ekh1 = params["ekh"]
    # s00 = vel^2*e2k0 + (e2s0 + og0^2*q3);  k3 = og0*q3 / s00
    f32 = lambda v: float(np.float32(v))
    c0 = f32(np.float32(e2s0) + np.float32(og0) * np.float32(og0) * np.float32(q3))
    c1 = f32(np.float32(e2s1) + np.float32(og1) * np.float32(og1) * np.float32(q4))
    ogq0 = f32(np.float32(og0) * np.float32(q3))
    ogq1 = f32(np.float32(og1) * np.float32(q4))
    qe0, qe1, qe2 = f32(q0 + eps), f32(q1 + eps), f32(q2 + eps)
    qe3, qe4 = f32(q3 + eps), f32(q4 + eps)
    half_pi = f32(_PI / 2)

    rows = 128 * C * T
    nc = bass.Bass()
    inp = nc.dram_tensor("inp", (rows, N_IN), F32, kind="ExternalInput")
    out_d = nc.dram_tensor("out_d", (rows, N_OD), F32, kind="ExternalOutput")
    out_a = nc.dram_tensor("out_a", (rows, N_OA), F32, kind="ExternalOutput")

    # Register const APs: for float biases of non-Copy activations, and for
    # tensor_scalar second operands (2-immediate TS instructions overflow the
    # ISA sync-wait slots, so scalars go through const APs instead).
    MAGIC = 12582912.0  # 1.5 * 2**23: fp32 round-to-nearest-integer trick
    inv2pi = f32(1.0 / (2 * math.pi))
    two_pi = f32(2 * math.pi)
    for val in {half_pi, qe0, qe1, qe2, c0, c1,
                MAGIC, inv2pi, -1.0, 1.0, -_PI, _PI}:
        val = f32(val)
        if (F32, val) not in nc.const_aps.aps:
            th = nc.alloc_sbuf_tensor(f"const-f32-{val}", [128, 1], F32)
            nc.gpsimd.memset(th.ap(), val)
            nc.const_aps.aps[(F32, val)] = th.ap()
    nc.all_engine_barrier()
    cap = lambda v: nc.const_aps.aps[(F32, f32(v))]

    inr = inp[:].rearrange("(t p c) f -> t p (c f)", t=T, p=128, c=C)
    odr = out_d[:].rearrange("(t p c) f -> t p (c f)", t=T, p=128, c=C)
    oar = out_a[:].rearrange("(t p c) f -> t p (c f)", t=T, p=128, c=C)

    with TileContext(nc) as tc:
        with tc.tile_pool(name="io", bufs=2) as io, \
             tc.tile_pool(name="scr", bufs=2) as scr:
            for t in range(T):
                it = io.tile([128, N_IN * C], F32, tag="it")
                nc.gpsimd.dma_start(it[:], inr[t])
                iv = it[:].rearrange("p (c f) -> p c f", f=N_IN)
                x2, a0, a1 = iv[:, :, 0], iv[:, :, 1], iv[:, :, 2]
                x0, x1 = iv[:, :, 3], iv[:, :, 4]
                p00, p11 = iv[:, :, 5], iv[:, :, 6]
                p02, p12, p22 = iv[:, :, 7], iv[:, :, 8], iv[:, :, 9]
                p01 = iv[:, :, 10]
                y0, y1 = iv[:, :, 11], iv[:, :, 12]

                od = io.tile([128, N_OD * C], F32, tag="od")
                dv = od[:].rearrange("p (c f) -> p c f", f=N_OD)
                oANGp = dv[:, :, 0:2]          # (angc, ang)
                oPXY = dv[:, :, 2:4]
                oXN = dv[:, :, 4:6]
                oB01, oB02, oB12 = dv[:, :, 6], dv[:, :, 7], dv[:, :, 8]

                oa = io.tile([128, N_OA * C], F32, tag="oa")
                av = oa[:].rearrange("p (c f) -> p c f", f=N_OA)
                oK = av[:, :, 0:2]
                oB00, oB11, oB22 = av[:, :, 2], av[:, :, 3], av[:, :, 4]
                oPN33, oPN44 = av[:, :, 5], av[:, :, 6]

                # --- angle: ang = wrap(x2 + angvel*dt); angc = wrap(+pi/2)
                # wrap(y) = y - 2pi*rint(y/2pi); rint via the 1.5*2^23 trick
                PRE = scr.tile([128, C, 2], F32, tag="PRE")
                KF = scr.tile([128, C, 2], F32, tag="KF")
                nc.vector.scalar_tensor_tensor(
                    PRE[:, :, 1], a1, g1 * dt, x2, OP.mult, OP.add)
                nc.vector.tensor_scalar_add(PRE[:, :, 0], PRE[:, :, 1],
                                            half_pi)
                nc.vector.tensor_scalar(KF[:, :, :], PRE[:, :, :],
                                        cap(inv2pi), cap(MAGIC),
                                        OP.mult, OP.add)
                nc.vector.tensor_scalar(KF[:, :, :], KF[:, :, :], cap(MAGIC),
                                        None, OP.subtract)
                nc.vector.scalar_tensor_tensor(
                    oANGp, KF[:, :, :], -two_pi, PRE[:, :, :],
                    OP.mult, OP.add)
                # guard against half-ulp overshoot past +/-pi (Sin range)
                nc.vector.tensor_scalar(oANGp, oANGp, cap(-_PI), cap(_PI),
                                        OP.max, OP.min)
                CS = scr.tile([128, C, 2], F32, tag="CS")   # (cos, sin)
                nc.scalar.activation(CS[:, :, :], oANGp, ACTF.Sin)

                # --- vel/angvel (ACT)
                VEL = scr.tile([128, C, 2], F32, tag="VEL")
                nc.scalar.mul(VEL[:, :, 0], a0, g0)
                nc.scalar.mul(VEL[:, :, 1], a1, g1)

                # --- j1 = vel*cos*dt (VCD); -j0 = vel*sin*dt (VSD)
                VCD = scr.tile([128, C], F32, tag="VCD")
                VSD = scr.tile([128, C], F32, tag="VSD")
                nc.vector.scalar_tensor_tensor(
                    VCD[:], CS[:, :, 0], dt, VEL[:, :, 0], OP.mult, OP.mult)
                nc.vector.scalar_tensor_tensor(
                    VSD[:], CS[:, :, 1], dt, VEL[:, :, 0], OP.mult, OP.mult)

                # --- px/py = clip(x + vel*trig*dt, -1, 1)
                PP = scr.tile([128, C, 2], F32, tag="PP")
                nc.vector.tensor_add(PP[:, :, 0], VCD[:], x0)
                nc.vector.tensor_add(PP[:, :, 1], VSD[:], x1)
                nc.vector.tensor_scalar(oPXY, PP[:, :, :], cap(-1.0), cap(1.0),
                                        OP.max, OP.min)

                # --- 3x3 covariance block  (j0 = -VSD, j1 = VCD)
                M1 = scr.tile([128, C], F32, tag="M1")
                M2 = scr.tile([128, C], F32, tag="M2")
                nc.vector.tensor_mul(M1[:], VSD[:], p22)
                nc.vector.tensor_sub(oB02, p02, M1[:])      # p02 + j0*p22
                nc.vector.tensor_mul(M2[:], VCD[:], p22)
                nc.vector.tensor_add(oB12, M2[:], p12)      # p12 + j1*p22
                nc.scalar.add(oB22, p22, qe2)

                E1 = scr.tile([128, C], F32, tag="E1")
                F1 = scr.tile([128, C], F32, tag="F1")
                G1 = scr.tile([128, C], F32, tag="G1")
                nc.vector.tensor_add(E1[:], oB02, p02)
                nc.vector.tensor_mul(F1[:], VSD[:], E1[:])
                nc.vector.tensor_sub(G1[:], p00, F1[:])     # p00 + j0*(B02+p02)
                nc.scalar.add(oB00, G1[:], qe0)

                E2 = scr.tile([128, C], F32, tag="E2")
                F2 = scr.tile([128, C], F32, tag="F2")
                G2 = scr.tile([128, C], F32, tag="G2")
                nc.vector.tensor_add(E2[:], oB12, p12)
                nc.vector.tensor_mul(F2[:], VCD[:], E2[:])
                nc.vector.tensor_add(G2[:], p11, F2[:])     # p11 + j1*(B12+p12)
                nc.scalar.add(oB11, G2[:], qe1)

                A1 = scr.tile([128, C], F32, tag="A1")
                A2 = scr.tile([128, C], F32, tag="A2")
                A3 = scr.tile([128, C], F32, tag="A3")
                nc.vector.tensor_mul(A1[:], VSD[:], oB12)   # -j0*B12
                nc.vector.tensor_mul(A2[:], VCD[:], p02)    # j1*p02
                nc.vector.tensor_sub(A3[:], A2[:], A1[:])   # j1*p02 + j0*B12
                nc.vector.tensor_add(oB01, A3[:], p01)

                # --- Kalman channel updates
                SQ = scr.tile([128, C, 2], F32, tag="SQ")
                S2 = scr.tile([128, C, 2], F32, tag="S2")
                R2 = scr.tile([128, C, 2], F32, tag="R2")
                nc.scalar.activation(SQ[:, :, 0], VEL[:, :, 0], ACTF.Square,
                                     scale=ekh0)
                nc.scalar.activation(SQ[:, :, 1], VEL[:, :, 1], ACTF.Square,
                                     scale=ekh1)
                nc.scalar.add(S2[:, :, 0], SQ[:, :, 0], c0)
                nc.scalar.add(S2[:, :, 1], SQ[:, :, 1], c1)
                nc.vector.reciprocal(R2[:, :, :], S2[:, :, :])
                nc.scalar.mul(av[:, :, 0], R2[:, :, 0], ogq0)   # k3
                nc.scalar.mul(av[:, :, 1], R2[:, :, 1], ogq1)   # k4

                ER = scr.tile([128, C, 2], F32, tag="ER")
                T2 = scr.tile([128, C, 2], F32, tag="T2")
                nc.vector.scalar_tensor_tensor(
                    ER[:, :, 0], a0, -(og0 * g0), y0, OP.mult, OP.add)
                nc.vector.scalar_tensor_tensor(
                    ER[:, :, 1], a1, -(og1 * g1), y1, OP.mult, OP.add)
                nc.vector.tensor_mul(T2[:, :, :], oK, ER[:, :, :])
                nc.vector.tensor_add(oXN, T2[:, :, :], VEL[:, :, :])

                nc.scalar.activation(oPN33, av[:, :, 0], ACTF.Copy,
                                     bias=qe3, scale=-ogq0)
                nc.scalar.activation(oPN44, av[:, :, 1], ACTF.Copy,
                                     bias=qe4, scale=-ogq1)

                nc.gpsimd.dma_start(odr[t], od[:])
                nc.gpsimd.dma_start(oar[t], oa[:])

    _split_excess_waits(nc)
    return nc


def _split_excess_waits(nc, keep=1):
    """Walrus's ISA structs here afford very few sem-wait slots per
    instruction. Hoist all but `keep` waits of any instruction onto
    same-engine NOPs inserted right before it (same queue => program order
    guarantees the waits still gate the instruction)."""
    import concourse.mybir as mybir

    ctr = 0
    for f in nc.m.functions:
        for b in f.blocks:
            new_insts = []
            for inst in b.instructions:
                si = getattr(inst, "sync_info", None)
                waits = list(si.on_wait) if (si is not None and si.on_wait) else []
                if len(waits) > keep:
                    excess, kept = waits[:-keep], waits[-keep:]
                    for w in excess:
                        nop = mybir.InstNoOp(name=f"waitnop-{ctr}", ins=[],
                                             outs=[])
                        ctr += 1
                        nop.engine = inst.engine
                        nop.sync_info = mybir.SyncInfo(on_update=[],
                                                       on_wait=[w])
                        new_insts.append(nop)
                    si.on_wait = kept
                new_insts.append(inst)
            b.instructions[:] = new_insts


def pack_inputs(x, P, a, Y, rows_per_core=ROWS_PER_CORE, n_cores=N_CORES):
    """Pack full inputs into per-core (rows_per_core, 13) fp32 records."""
    n = x.shape[0]
    per = n // n_cores
    src = np.empty((n, N_IN), np.float32)
    src[:, 0] = x[:, 2]
    src[:, 1] = a[:, 0]
    src[:, 2] = a[:, 1]
    src[:, 3] = x[:, 0]
    src[:, 4] = x[:, 1]
    Pf = P.reshape(n, 25)
    src[:, 5] = Pf[:, 0]    # p00
    src[:, 6] = Pf[:, 6]    # p11
    src[:, 7] = Pf[:, 2]    # p02
    src[:, 8] = Pf[:, 7]    # p12
    src[:, 9] = Pf[:, 12]   # p22
    src[:, 10] = Pf[:, 1]   # p01
    src[:, 11] = Y[0, :]
    src[:, 12] = Y[1, :]
    packed = np.zeros((n_cores, rows_per_core, N_IN), np.float32)
    for k in range(n_cores):
        packed[k, :per] = src[k * per:(k + 1) * per]
    return packed


def unpack_outputs(outs_d, outs_a, n, per):
    """Per-core (rows,9)+(rows,7) record arrays -> (x_new, P_new, K)."""
    d = np.concatenate([ob[:per] for ob in outs_d], axis=0)
    a_ = np.concatenate([ob[:per] for ob in outs_a], axis=0)
    assert d.shape[0] == n and a_.shape[0] == n
    x_new = np.empty((n, 5), np.float32)
    x_new[:, 0] = d[:, 2]   # px
    x_new[:, 1] = d[:, 3]   # py
    x_new[:, 2] = d[:, 1]   # ang
    x_new[:, 3] = d[:, 4]   # xn3
    x_new[:, 4] = d[:, 5]   # xn4
    P_new = np.zeros((n, 25), np.float32)
    P_new[:, 0] = a_[:, 2]                   # [0,0] B00
    P_new[:, 1] = P_new[:, 5] = d[:, 6]      # [0,1],[1,0] B01
    P_new[:, 2] = P_new[:, 10] = d[:, 7]     # [0,2],[2,0] B02
    P_new[:, 6] = a_[:, 3]                   # [1,1] B11
    P_new[:, 7] = P_new[:, 11] = d[:, 8]     # [1,2],[2,1] B12
    P_new[:, 12] = a_[:, 4]                  # [2,2] B22
    P_new[:, 18] = a_[:, 5]                  # [3,3]
    P_new[:, 24] = a_[:, 6]                  # [4,4]
    P_new = P_new.reshape(n, 5, 5)
    K = np.zeros((n, 10), np.float32)
    K[:, 6] = a_[:, 0]                       # [3,0] k3
    K[:, 9] = a_[:, 1]                       # [4,1] k4
    K = K.reshape(n, 5, 2)
    return x_new, P_new, K


_CACHE = {}
_TRACE = False        # test harness can set kernel._TRACE = True
_LAST_RESULT = None   # BassKernelResults of the most recent run


def _get_program(params):
    key = tuple(sorted((k, tuple(v) if isinstance(v, list) else v)
                       for k, v in params.items()))
    if key not in _CACHE:
        _CACHE[key] = build_program(params)
    return _CACHE[key]


def kernel(x, P, a, Y, n1, n2_sigma, n2_kappa, gains, obs_gains):
    from concourse.bass_utils import run_bass_kernel_spmd

    x = np.asarray(x, np.float32)
    P = np.asarray(P, np.float32)
    a = np.asarray(a, np.float32)
    Y = np.asarray(Y, np.float32)
    n = x.shape[0]
    assert n == N_TOTAL and n % N_CORES == 0

    params = _params_from_inputs(n1, n2_sigma, n2_kappa, gains, obs_gains)
    nc = _get_program(params)

    packed = pack_inputs(x, P, a, Y)
    in_maps = [{"inp": np.ascontiguousarray(packed[k])} for k in range(N_CORES)]
    res = run_bass_kernel_spmd(nc, in_maps, core_ids=list(range(N_CORES)),
                               trace=_TRACE)
    global _LAST_RESULT
    _LAST_RESULT = res
    outs_d = [r["out_d"] for r in res.results]
    outs_a = [r["out_a"] for r in res.results]
    return unpack_outputs(outs_d, outs_a, n, n // N_CORES)


# revision 18
# speedup vs baseline: 1.1191x; 1.1191x over previous
"""Trainium2 Bass kernel for nn_BatchBeliefStep (batched EKF step).

Exploits structure of the reference: the Jacobian A has zero rows/cols 3,4,
so P_new is block-diagonal (3x3 block + two diagonal entries), K has only
2 nonzero entries per row, and the innovation covariance S is diagonal.

Per row only 13 input floats and 15 output floats matter; the host packs
these into dense plane-major records (each feature a contiguous (128, C)
block per tile), the device runs a pure elementwise kernel over them
(data-parallel across 8 cores), and the host scatters the results back
into the full-shape outputs.
"""

import math

import numpy as np

N_TOTAL = 500_000
N_CORES = 8
TILE_C = 245          # records per partition per tile
TILE_T = 2            # tiles per core
ROWS_PER_CORE = 128 * TILE_C * TILE_T  # 62720 (62500 real + 220 pad)

N_IN = 13             # packed input features per row
N_OD = 9              # DVE-written output features per row
N_OA = 7              # ACT/GPSIMD-written output features per row

_PI = float(np.float32(math.pi))


def _params_from_inputs(n1, n2_sigma, n2_kappa, gains, obs_gains):
    f = lambda v: float(np.float32(v))
    q = [f(np.exp(np.float32(2.0) * np.float32(v))) for v in np.asarray(n1)]
    e2s = [f(np.exp(np.float32(2.0) * np.float32(v))) for v in np.asarray(n2_sigma)]
    ekh = [f(np.exp(np.float32(v))) for v in np.asarray(n2_kappa)]
    g = [f(v) for v in np.asarray(gains)]
    og = [f(v) for v in np.asarray(obs_gains)]
    return dict(q=q, e2s=e2s, ekh=ekh, g=g, og=og, dt=0.1, eps=1e-6)


def build_program(params, C=TILE_C, T=TILE_T, split_waits=True):
    """Build the Bass program (plane-major layout).

    inp   (T, 128, 13*C): features [x2, a0, a1, x0, x1, p00, p11, p02,
                          p12, p22, p01, y0, y1], each a length-C segment
    out_d (T, 128, 9*C):  [angc*, ang, px, py, xn3, xn4, B01, B02, B12]
    out_a (T, 128, 7*C):  [k3, k4, B00, B11, B22, pn33, pn44]
    (* = dummy; cos-angle slot is only the Sin input)
    """
    import concourse.bass as bass
    import concourse.mybir as mybir
    from concourse.tile import TileContext

    F32 = mybir.dt.float32
    OP = mybir.AluOpType
    ACTF = mybir.ActivationFunctionType

    dt = params["dt"]
    eps = params["eps"]
    g0, g1 = params["g"]
    og0, og1 = params["og"]
    q0, q1, q2, q3, q4 = params["q"]
    e2s0, e2s1 = params["e2s"]
    ekh0, ekh1 = params["ekh"]
    # s00 = vel^2*e2k0 + (e2s0 + og0^2*q3);  k3 = og0*q3 / s00
    f32 = lambda v: float(np.float32(v))
    c0 = f32(np.float32(e2s0) + np.float32(og0) * np.float32(og0) * np.float32(q3))
    c1 = f32(np.float32(e2s1) + np.float32(og1) * np.float32(og1) * np.float32(q4))
    ogq0 = f32(np.float32(og0) * np.float32(q3))
    ogq1 = f32(np.float32(og1) * np.float32(q4))
    qe0, qe1, qe2 = f32(q0 + eps), f32(q1 + eps), f32(q2 + eps)
    qe3, qe4 = f32(q3 + eps), f32(q4 + eps)
    half_pi = f32(_PI / 2)
    MAGIC = 12582912.0  # 1.5 * 2**23: fp32 round-to-nearest-integer trick
    inv2pi = f32(1.0 / (2 * math.pi))
    two_pi = f32(2 * math.pi)

    nc = bass.Bass()
    inp = nc.dram_tensor("inp", (T, 128, N_IN * C), F32, kind="ExternalInput")
    out_d = nc.dram_tensor("out_d", (T, 128, N_OD * C), F32,
                           kind="ExternalOutput")
    out_a = nc.dram_tensor("out_a", (T, 128, N_OA * C), F32,
                           kind="ExternalOutput")

    with TileContext(nc) as tc:
        with tc.tile_pool(name="cst", bufs=1) as cst, \
             tc.tile_pool(name="io", bufs=2) as io, \
             tc.tile_pool(name="scr", bufs=2) as scr:
            # Const scalar APs ((128,1) tiles) for activation biases and
            # tensor_scalar operands (2-immediate TS instructions overflow
            # the ISA sync-wait slots, so scalars go via APs instead).
            for val in {half_pi, qe0, qe1, qe2, c0, c1, MAGIC, inv2pi,
                        -1.0, 1.0, -_PI, _PI}:
                val = f32(val)
                if (F32, val) not in nc.const_aps.aps:
                    ct = cst.tile([128, 1], F32, name=f"c{len(nc.const_aps.aps)}")
                    nc.gpsimd.memset(ct[:], val)
                    nc.const_aps.aps[(F32, val)] = ct[:]
            cap = lambda v: nc.const_aps.aps[(F32, f32(v))]

            for t in range(T):
                it = io.tile([128, N_IN, C], F32, tag="it")
                nc.sync.dma_start(it[:], inp[t])
                x2, a0, a1 = it[:, 0, :], it[:, 1, :], it[:, 2, :]
                x0, x1 = it[:, 3, :], it[:, 4, :]
                p00, p11 = it[:, 5, :], it[:, 6, :]
                p02, p12, p22 = it[:, 7, :], it[:, 8, :], it[:, 9, :]
                p01 = it[:, 10, :]
                y0, y1 = it[:, 11, :], it[:, 12, :]

                od = io.tile([128, N_OD, C], F32, tag="od")
                oANGp = od[:, 0:2, :]          # (angc, ang)
                oPXY = od[:, 2:4, :]
                oXN = od[:, 4:6, :]
                oB01, oB02, oB12 = od[:, 6, :], od[:, 7, :], od[:, 8, :]

                oa = io.tile([128, N_OA, C], F32, tag="oa")
                oK = oa[:, 0:2, :]
                oB00, oB11, oB22 = oa[:, 2, :], oa[:, 3, :], oa[:, 4, :]
                oPN33, oPN44 = oa[:, 5, :], oa[:, 6, :]

                # scratch (feature-slot layout, all unit-stride views)
                SA = scr.tile([128, 8, C], F32, tag="SA")
                PRE = SA[:, 0:2, :]            # (angc_pre, ang_pre)
                KF = SA[:, 2:4, :]
                CS = SA[:, 4:6, :]             # (cos, sin)
                VEL = SA[:, 6:8, :]            # (vel, angvel)
                SB = scr.tile([128, 8, C], F32, tag="SB")
                VCD, VSD = SB[:, 0, :], SB[:, 1, :]
                PP = SB[:, 2:4, :]
                M1, M2 = SB[:, 4, :], SB[:, 5, :]
                E1, F1 = SB[:, 6, :], SB[:, 7, :]
                SC = scr.tile([128, 7, C], F32, tag="SC")
                G1, E2, F2, G2 = (SC[:, 0, :], SC[:, 1, :], SC[:, 2, :],
                                  SC[:, 3, :])
                A1, A2, A3 = SC[:, 4, :], SC[:, 5, :], SC[:, 6, :]
                SD = scr.tile([128, 6, C], F32, tag="SD")
                SQ = SD[:, 0:2, :]
                ER = SD[:, 2:4, :]
                T2 = SD[:, 4:6, :]

                # --- angle: ang = wrap(x2 + angvel*dt); angc = wrap(+pi/2)
                # wrap(y) = y - 2pi*rint(y/2pi); rint via the MAGIC trick
                nc.vector.scalar_tensor_tensor(
                    PRE[:, 1, :], a1, g1 * dt, x2, OP.mult, OP.add)
                nc.vector.tensor_scalar_add(PRE[:, 0, :], PRE[:, 1, :],
                                            half_pi)
                nc.vector.tensor_scalar(KF, PRE, cap(inv2pi), cap(MAGIC),
                                        OP.mult, OP.add)
                nc.vector.tensor_scalar(KF, KF, cap(MAGIC), None, OP.subtract)
                nc.vector.scalar_tensor_tensor(oANGp, KF, -two_pi, PRE,
                                               OP.mult, OP.add)
                # guard against half-ulp overshoot past +/-pi (Sin range)
                nc.vector.tensor_scalar(oANGp, oANGp, cap(-_PI), cap(_PI),
                                        OP.max, OP.min)
                nc.scalar.activation(CS, oANGp, ACTF.Sin)

                # --- vel/angvel (ACT)
                nc.scalar.mul(VEL[:, 0, :], a0, g0)
                nc.scalar.mul(VEL[:, 1, :], a1, g1)

                # --- j1 = vel*cos*dt (VCD); -j0 = vel*sin*dt (VSD)
                nc.vector.scalar_tensor_tensor(
                    VCD, CS[:, 0, :], dt, VEL[:, 0, :], OP.mult, OP.mult)
                nc.vector.scalar_tensor_tensor(
                    VSD, CS[:, 1, :], dt, VEL[:, 0, :], OP.mult, OP.mult)

                # --- px/py = clip(x + vel*trig*dt, -1, 1)
                nc.vector.tensor_add(PP[:, 0, :], VCD, x0)
                nc.vector.tensor_add(PP[:, 1, :], VSD, x1)
                nc.vector.tensor_scalar(oPXY, PP, cap(-1.0), cap(1.0),
                                        OP.max, OP.min)

                # --- 3x3 covariance block  (j0 = -VSD, j1 = VCD)
                nc.vector.tensor_mul(M1, VSD, p22)
                nc.vector.tensor_sub(oB02, p02, M1)         # p02 + j0*p22
                nc.vector.tensor_mul(M2, VCD, p22)
                nc.vector.tensor_add(oB12, M2, p12)         # p12 + j1*p22
                nc.scalar.add(oB22, p22, qe2)

                nc.vector.tensor_add(E1, oB02, p02)
                nc.vector.tensor_mul(F1, VSD, E1)
                nc.vector.tensor_sub(G1, p00, F1)           # p00 + j0*(B02+p02)
                nc.scalar.add(oB00, G1, qe0)

                nc.vector.tensor_add(E2, oB12, p12)
                nc.vector.tensor_mul(F2, VCD, E2)
                nc.vector.tensor_add(G2, p11, F2)           # p11 + j1*(B12+p12)
                nc.scalar.add(oB11, G2, qe1)

                nc.vector.tensor_mul(A1, VSD, oB12)         # -j0*B12
                nc.vector.tensor_mul(A2, VCD, p02)          # j1*p02
                nc.vector.tensor_sub(A3, A2, A1)            # j1*p02 + j0*B12
                nc.vector.tensor_add(oB01, A3, p01)

                # --- Kalman channels: s00, s11; k = og*q*s_other/(s00*s11)
                nc.scalar.activation(SQ[:, 0, :], VEL[:, 0, :], ACTF.Square,
                                     scale=ekh0)
                nc.scalar.activation(SQ[:, 1, :], VEL[:, 1, :], ACTF.Square,
                                     scale=ekh1)
                nc.scalar.add(SQ[:, 0, :], SQ[:, 0, :], c0)
                nc.scalar.add(SQ[:, 1, :], SQ[:, 1, :], c1)
                PRD, IP = SC[:, 1, :], SC[:, 2, :]   # reuse E2/F2 slots
                nc.vector.tensor_mul(PRD, SQ[:, 0, :], SQ[:, 1, :])
                nc.vector.reciprocal(IP, PRD)
                nc.vector.scalar_tensor_tensor(
                    oa[:, 0, :], SQ[:, 1, :], ogq0, IP, OP.mult, OP.mult)
                nc.vector.scalar_tensor_tensor(
                    oa[:, 1, :], SQ[:, 0, :], ogq1, IP, OP.mult, OP.mult)

                nc.vector.scalar_tensor_tensor(
                    ER[:, 0, :], a0, -(og0 * g0), y0, OP.mult, OP.add)
                nc.vector.scalar_tensor_tensor(
                    ER[:, 1, :], a1, -(og1 * g1), y1, OP.mult, OP.add)
                nc.vector.tensor_mul(T2, oK, ER)
                nc.vector.tensor_add(oXN, T2, VEL)

                nc.scalar.activation(oPN33, oa[:, 0, :], ACTF.Copy,
                                     bias=qe3, scale=-ogq0)
                nc.scalar.activation(oPN44, oa[:, 1, :], ACTF.Copy,
                                     bias=qe4, scale=-ogq1)

                nc.sync.dma_start(out_d[t], od[:])
                nc.sync.dma_start(out_a[t], oa[:])

    if split_waits:
        _split_excess_waits(nc)
    return nc


def _split_excess_waits(nc, keep=1):
    """Walrus's ISA structs here afford very few sem-wait slots per
    instruction. Hoist all but `keep` waits of any instruction onto
    same-engine NOPs inserted right before it (same queue => program order
    guarantees the waits still gate the instruction)."""
    import concourse.mybir as mybir

    ctr = 0
    for f in nc.m.functions:
        for b in f.blocks:
            new_insts = []
            for inst in b.instructions:
                si = getattr(inst, "sync_info", None)
                waits = list(si.on_wait) if (si is not None and si.on_wait) else []
                if len(waits) > keep:
                    excess, kept = waits[:-keep], waits[-keep:]
                    for w in excess:
                        nop = mybir.InstNoOp(name=f"waitnop-{ctr}", ins=[],
                                             outs=[])
                        ctr += 1
                        nop.engine = inst.engine
                        nop.sync_info = mybir.SyncInfo(on_update=[],
                                                       on_wait=[w])
                        new_insts.append(nop)
                    si.on_wait = kept
                new_insts.append(inst)
            b.instructions[:] = new_insts


_IN_PERM = None  # feature order in packed input


def pack_inputs(x, P, a, Y, rows_per_core=ROWS_PER_CORE, n_cores=N_CORES,
                C=TILE_C, T=TILE_T):
    """Pack full inputs into per-core (T, 128, 13*C) plane-major arrays."""
    n = x.shape[0]
    per = n // n_cores
    src = np.empty((n, N_IN), np.float32)
    src[:, 0] = x[:, 2]
    src[:, 1] = a[:, 0]
    src[:, 2] = a[:, 1]
    src[:, 3] = x[:, 0]
    src[:, 4] = x[:, 1]
    Pf = P.reshape(n, 25)
    src[:, 5] = Pf[:, 0]    # p00
    src[:, 6] = Pf[:, 6]    # p11
    src[:, 7] = Pf[:, 2]    # p02
    src[:, 8] = Pf[:, 7]    # p12
    src[:, 9] = Pf[:, 12]   # p22
    src[:, 10] = Pf[:, 1]   # p01
    src[:, 11] = Y[0, :]
    src[:, 12] = Y[1, :]
    packed = np.zeros((n_cores, rows_per_core, N_IN), np.float32)
    for k in range(n_cores):
        packed[k, :per] = src[k * per:(k + 1) * per]
    # rows-major (RPC, F) -> plane-major (T, 128, F, C)
    pm = packed.reshape(n_cores, T, 128, C, N_IN).transpose(0, 1, 2, 4, 3)
    return np.ascontiguousarray(pm).reshape(n_cores, T, 128, N_IN * C)


def _planes_to_rows(o, F, C=TILE_C, T=TILE_T):
    """(T, 128, F*C) plane-major -> (T*128*C, F) row records."""
    return o.reshape(T, 128, F, C).transpose(0, 1, 3, 2).reshape(-1, F)


def unpack_outputs(outs_d, outs_a, n, per):
    """Per-core plane-major outputs -> (x_new, P_new, K)."""
    d = np.concatenate(
        [_planes_to_rows(ob, N_OD)[:per] for ob in outs_d], axis=0)
    a_ = np.concatenate(
        [_planes_to_rows(ob, N_OA)[:per] for ob in outs_a], axis=0)
    assert d.shape[0] == n and a_.shape[0] == n
    x_new = np.empty((n, 5), np.float32)
    x_new[:, 0] = d[:, 2]   # px
    x_new[:, 1] = d[:, 3]   # py
    x_new[:, 2] = d[:, 1]   # ang
    x_new[:, 3] = d[:, 4]   # xn3
    x_new[:, 4] = d[:, 5]   # xn4
    P_new = np.zeros((n, 25), np.float32)
    P_new[:, 0] = a_[:, 2]                   # [0,0] B00
    P_new[:, 1] = P_new[:, 5] = d[:, 6]      # [0,1],[1,0] B01
    P_new[:, 2] = P_new[:, 10] = d[:, 7]     # [0,2],[2,0] B02
    P_new[:, 6] = a_[:, 3]                   # [1,1] B11
    P_new[:, 7] = P_new[:, 11] = d[:, 8]     # [1,2],[2,1] B12
    P_new[:, 12] = a_[:, 4]                  # [2,2] B22
    P_new[:, 18] = a_[:, 5]                  # [3,3]
    P_new[:, 24] = a_[:, 6]                  # [4,4]
    P_new = P_new.reshape(n, 5, 5)
    K = np.zeros((n, 10), np.float32)
    K[:, 6] = a_[:, 0]                       # [3,0] k3
    K[:, 9] = a_[:, 1]                       # [4,1] k4
    K = K.reshape(n, 5, 2)
    return x_new, P_new, K


_CACHE = {}
_TRACE = False        # test harness can set kernel._TRACE = True
_LAST_RESULT = None   # BassKernelResults of the most recent run


def _get_program(params):
    key = tuple(sorted((k, tuple(v) if isinstance(v, list) else v)
                       for k, v in params.items()))
    if key not in _CACHE:
        _CACHE[key] = build_program(params)
    return _CACHE[key]


def kernel(x, P, a, Y, n1, n2_sigma, n2_kappa, gains, obs_gains):
    from concourse.bass_utils import run_bass_kernel_spmd

    x = np.asarray(x, np.float32)
    P = np.asarray(P, np.float32)
    a = np.asarray(a, np.float32)
    Y = np.asarray(Y, np.float32)
    n = x.shape[0]
    assert n == N_TOTAL and n % N_CORES == 0

    params = _params_from_inputs(n1, n2_sigma, n2_kappa, gains, obs_gains)
    nc = _get_program(params)

    packed = pack_inputs(x, P, a, Y)
    in_maps = [{"inp": np.ascontiguousarray(packed[k])} for k in range(N_CORES)]
    res = run_bass_kernel_spmd(nc, in_maps, core_ids=list(range(N_CORES)),
                               trace=_TRACE)
    global _LAST_RESULT
    _LAST_RESULT = res
    outs_d = [r["out_d"] for r in res.results]
    outs_a = [r["out_a"] for r in res.results]
    return unpack_outputs(outs_d, outs_a, n, n // N_CORES)


# revision 24
# speedup vs baseline: 1.2036x; 1.0755x over previous
"""Trainium2 Bass kernel for nn_BatchBeliefStep (batched EKF step).

Exploits structure of the reference: the Jacobian A has zero rows/cols 3,4,
so P_new is block-diagonal (3x3 block + two diagonal entries), K has only
2 nonzero entries per row, and the innovation covariance S is diagonal.

Per row only 13 input floats and 15 output floats matter; the host packs
these into dense plane-major records (each feature a contiguous (128, C)
block per tile), the device runs a pure elementwise kernel over them
(data-parallel across 8 cores), and the host scatters the results back
into the full-shape outputs.
"""

import math

import numpy as np

N_TOTAL = 500_000
N_CORES = 8
TILE_C = 246          # records per partition per tile (even: DVE 2x mode)
TILE_T = 2            # tiles per core
ROWS_PER_CORE = 128 * TILE_C * TILE_T  # 62976 (62500 real + 476 pad)

N_IN = 13             # packed input features per row
N_OD = 9              # DVE-written output features per row
N_OA = 7              # ACT/GPSIMD-written output features per row

_PI = float(np.float32(math.pi))


def _params_from_inputs(n1, n2_sigma, n2_kappa, gains, obs_gains):
    f = lambda v: float(np.float32(v))
    q = [f(np.exp(np.float32(2.0) * np.float32(v))) for v in np.asarray(n1)]
    e2s = [f(np.exp(np.float32(2.0) * np.float32(v))) for v in np.asarray(n2_sigma)]
    ekh = [f(np.exp(np.float32(v))) for v in np.asarray(n2_kappa)]
    g = [f(v) for v in np.asarray(gains)]
    og = [f(v) for v in np.asarray(obs_gains)]
    return dict(q=q, e2s=e2s, ekh=ekh, g=g, og=og, dt=0.1, eps=1e-6)


def build_program(params, C=TILE_C, T=TILE_T, split_waits=True):
    """Build the Bass program (plane-major layout).

    inp   (T, 128, 13*C): features [x2, a0, a1, x0, x1, p00, p11, p02,
                          p12, p22, p01, y0, y1], each a length-C segment
    out_d (T, 128, 9*C):  [angc*, ang, px, py, xn3, xn4, B01, B02, B12]
    out_a (T, 128, 7*C):  [k3, k4, B00, B11, B22, pn33, pn44]
    (* = dummy; cos-angle slot is only the Sin input)
    """
    import concourse.bass as bass
    import concourse.mybir as mybir
    from concourse.tile import TileContext

    F32 = mybir.dt.float32
    OP = mybir.AluOpType
    ACTF = mybir.ActivationFunctionType

    dt = params["dt"]
    eps = params["eps"]
    g0, g1 = params["g"]
    og0, og1 = params["og"]
    q0, q1, q2, q3, q4 = params["q"]
    e2s0, e2s1 = params["e2s"]
    ekh0, ekh1 = params["ekh"]
    # s00 = vel^2*e2k0 + (e2s0 + og0^2*q3);  k3 = og0*q3 / s00
    f32 = lambda v: float(np.float32(v))
    c0 = f32(np.float32(e2s0) + np.float32(og0) * np.float32(og0) * np.float32(q3))
    c1 = f32(np.float32(e2s1) + np.float32(og1) * np.float32(og1) * np.float32(q4))
    ogq0 = f32(np.float32(og0) * np.float32(q3))
    ogq1 = f32(np.float32(og1) * np.float32(q4))
    qe0, qe1, qe2 = f32(q0 + eps), f32(q1 + eps), f32(q2 + eps)
    qe3, qe4 = f32(q3 + eps), f32(q4 + eps)
    half_pi = f32(_PI / 2)
    MAGIC = 12582912.0  # 1.5 * 2**23: fp32 round-to-nearest-integer trick
    inv2pi = f32(1.0 / (2 * math.pi))
    two_pi = f32(2 * math.pi)

    nc = bass.Bass()
    inp = nc.dram_tensor("inp", (T, 128, N_IN * C), F32, kind="ExternalInput")
    out_d = nc.dram_tensor("out_d", (T, 128, N_OD * C), F32,
                           kind="ExternalOutput")
    out_a = nc.dram_tensor("out_a", (T, 128, N_OA * C), F32,
                           kind="ExternalOutput")

    with TileContext(nc) as tc:
        with tc.tile_pool(name="io", bufs=2) as io, \
             tc.tile_pool(name="scr", bufs=2) as scr:

            for t in range(T):
                it = io.tile([128, N_IN, C], F32, tag="it")
                nc.sync.dma_start(it[:], inp[t])
                x2, a0, a1 = it[:, 0, :], it[:, 1, :], it[:, 2, :]
                x0, x1 = it[:, 3, :], it[:, 4, :]
                p00, p11 = it[:, 5, :], it[:, 6, :]
                p02, p12, p22 = it[:, 7, :], it[:, 8, :], it[:, 9, :]
                p01 = it[:, 10, :]
                y0, y1 = it[:, 11, :], it[:, 12, :]

                od = io.tile([128, N_OD, C], F32, tag="od")
                oANGp = od[:, 0:2, :]          # (angc, ang)
                oPXY = od[:, 2:4, :]
                oXN = od[:, 4:6, :]
                oB01, oB02, oB12 = od[:, 6, :], od[:, 7, :], od[:, 8, :]

                oa = io.tile([128, N_OA, C], F32, tag="oa")
                oK = oa[:, 0:2, :]
                oB00, oB11, oB22 = oa[:, 2, :], oa[:, 3, :], oa[:, 4, :]
                oPN33, oPN44 = oa[:, 5, :], oa[:, 6, :]

                # scratch (feature-slot layout, all unit-stride views)
                SA = scr.tile([128, 8, C], F32, tag="SA")
                PRE = SA[:, 0:2, :]            # (angc_pre, ang_pre)
                KF = SA[:, 2:4, :]
                CS = SA[:, 4:6, :]             # (cos, sin)
                VEL = SA[:, 6:8, :]            # (vel, angvel)
                SB = scr.tile([128, 8, C], F32, tag="SB")
                JJ = SB[:, 0:2, :]             # (j0, j1)
                PP = SB[:, 2:4, :]
                M12 = SB[:, 4:6, :]
                E12 = SB[:, 6:8, :]
                SC = scr.tile([128, 8, C], F32, tag="SC")
                F12 = SC[:, 0:2, :]
                G12 = SC[:, 2:4, :]
                A1, A2, A3 = SC[:, 4, :], SC[:, 5, :], SC[:, 6, :]
                PRD, IP = SC[:, 7, :], SB[:, 4, :]   # IP reuses M12[0] slot
                SD = scr.tile([128, 6, C], F32, tag="SD")
                SQ = SD[:, 0:2, :]
                ER = SD[:, 2:4, :]
                T2 = SD[:, 4:6, :]

                # --- angle: ang = wrap(x2 + angvel*dt); angc = wrap(+pi/2)
                # wrap(y) = y - 2pi*rint(y/2pi); rint via the MAGIC trick
                nc.vector.scalar_tensor_tensor(
                    PRE[:, 1, :], a1, g1 * dt, x2, OP.mult, OP.add)
                nc.scalar.activation(PRE[:, 0, :], PRE[:, 1, :], ACTF.Copy,
                                     bias=half_pi)
                nc.scalar.activation(KF, PRE, ACTF.Copy, bias=MAGIC,
                                     scale=inv2pi)
                nc.scalar.activation(KF, KF, ACTF.Copy, bias=-MAGIC)
                nc.vector.scalar_tensor_tensor(oANGp, KF, -two_pi, PRE,
                                               OP.mult, OP.add)
                # guard against half-ulp overshoot past +/-pi (Sin range)
                nc.vector.tensor_scalar(oANGp, oANGp, -_PI, _PI,
                                        OP.max, OP.min)
                nc.scalar.activation(CS, oANGp, ACTF.Sin)

                # --- vel/angvel (ACT)
                nc.scalar.mul(VEL[:, 0, :], a0, g0)
                nc.scalar.mul(VEL[:, 1, :], a1, g1)

                # --- j0 = -vel*sin*dt; j1 = vel*cos*dt
                nc.vector.scalar_tensor_tensor(
                    JJ[:, 0, :], CS[:, 1, :], -dt, VEL[:, 0, :],
                    OP.mult, OP.mult)
                nc.vector.scalar_tensor_tensor(
                    JJ[:, 1, :], CS[:, 0, :], dt, VEL[:, 0, :],
                    OP.mult, OP.mult)

                # --- px = clip(x0 + j1), py = clip(x1 - j0)
                nc.vector.tensor_add(PP[:, 0, :], JJ[:, 1, :], x0)
                nc.vector.tensor_sub(PP[:, 1, :], x1, JJ[:, 0, :])
                nc.vector.tensor_scalar(oPXY, PP, -1.0, 1.0,
                                        OP.max, OP.min)

                # --- 3x3 covariance block, channels paired over (j0, j1)
                P0212 = it[:, 7:9, :]
                P0011 = it[:, 5:7, :]
                p22b = it[:, 9:10, :].to_broadcast((128, 2, C))
                nc.vector.tensor_mul(M12, JJ, p22b)          # (j0, j1)*p22
                nc.vector.tensor_add(od[:, 7:9, :], M12, P0212)   # B02, B12
                nc.scalar.activation(oB22, p22, ACTF.Copy, bias=qe2)

                nc.vector.tensor_add(E12, od[:, 7:9, :], P0212)
                nc.vector.tensor_mul(F12, JJ, E12)
                nc.vector.tensor_add(G12, P0011, F12)
                nc.scalar.activation(oB00, G12[:, 0, :], ACTF.Copy, bias=qe0)
                nc.scalar.activation(oB11, G12[:, 1, :], ACTF.Copy, bias=qe1)

                nc.vector.tensor_mul(A1, JJ[:, 0, :], oB12)  # j0*B12
                nc.vector.tensor_mul(A2, JJ[:, 1, :], p02)   # j1*p02
                nc.vector.tensor_add(A3, A2, A1)
                nc.vector.tensor_add(oB01, A3, p01)

                # --- Kalman channels: s00, s11; k = og*q*s_other/(s00*s11)
                nc.scalar.activation(SQ[:, 0, :], VEL[:, 0, :], ACTF.Square,
                                     scale=ekh0)
                nc.scalar.activation(SQ[:, 1, :], VEL[:, 1, :], ACTF.Square,
                                     scale=ekh1)
                nc.scalar.activation(SQ[:, 0, :], SQ[:, 0, :], ACTF.Copy,
                                     bias=c0)
                nc.scalar.activation(SQ[:, 1, :], SQ[:, 1, :], ACTF.Copy,
                                     bias=c1)
                nc.vector.tensor_mul(PRD, SQ[:, 0, :], SQ[:, 1, :])
                nc.vector.reciprocal(IP, PRD)
                nc.vector.scalar_tensor_tensor(
                    oa[:, 0, :], SQ[:, 1, :], ogq0, IP, OP.mult, OP.mult)
                nc.vector.scalar_tensor_tensor(
                    oa[:, 1, :], SQ[:, 0, :], ogq1, IP, OP.mult, OP.mult)

                nc.vector.scalar_tensor_tensor(
                    ER[:, 0, :], a0, -(og0 * g0), y0, OP.mult, OP.add)
                nc.vector.scalar_tensor_tensor(
                    ER[:, 1, :], a1, -(og1 * g1), y1, OP.mult, OP.add)
                nc.vector.tensor_mul(T2, oK, ER)
                nc.vector.tensor_add(oXN, T2, VEL)

                nc.scalar.activation(oPN33, oa[:, 0, :], ACTF.Copy,
                                     bias=qe3, scale=-ogq0)
                nc.scalar.activation(oPN44, oa[:, 1, :], ACTF.Copy,
                                     bias=qe4, scale=-ogq1)

                nc.sync.dma_start(out_d[t], od[:])
                nc.sync.dma_start(out_a[t], oa[:])

    if split_waits:
        _split_excess_waits(nc)
    return nc


def _split_excess_waits(nc, keep=1):
    """Walrus's ISA structs here afford very few sem-wait slots per
    instruction. Hoist all but `keep` waits of any instruction onto
    same-engine NOPs inserted right before it (same queue => program order
    guarantees the waits still gate the instruction)."""
    import concourse.mybir as mybir

    ctr = 0
    for f in nc.m.functions:
        for b in f.blocks:
            new_insts = []
            for inst in b.instructions:
                si = getattr(inst, "sync_info", None)
                waits = list(si.on_wait) if (si is not None and si.on_wait) else []
                if len(waits) > keep:
                    excess, kept = waits[:-keep], waits[-keep:]
                    for w in excess:
                        nop = mybir.InstNoOp(name=f"waitnop-{ctr}", ins=[],
                                             outs=[])
                        ctr += 1
                        nop.engine = inst.engine
                        nop.sync_info = mybir.SyncInfo(on_update=[],
                                                       on_wait=[w])
                        new_insts.append(nop)
                    si.on_wait = kept
                new_insts.append(inst)
            b.instructions[:] = new_insts


_IN_PERM = None  # feature order in packed input


def pack_inputs(x, P, a, Y, rows_per_core=ROWS_PER_CORE, n_cores=N_CORES,
                C=TILE_C, T=TILE_T):
    """Pack full inputs into per-core (T, 128, 13*C) plane-major arrays."""
    n = x.shape[0]
    per = n // n_cores
    src = np.empty((n, N_IN), np.float32)
    src[:, 0] = x[:, 2]
    src[:, 1] = a[:, 0]
    src[:, 2] = a[:, 1]
    src[:, 3] = x[:, 0]
    src[:, 4] = x[:, 1]
    Pf = P.reshape(n, 25)
    src[:, 5] = Pf[:, 0]    # p00
    src[:, 6] = Pf[:, 6]    # p11
    src[:, 7] = Pf[:, 2]    # p02
    src[:, 8] = Pf[:, 7]    # p12
    src[:, 9] = Pf[:, 12]   # p22
    src[:, 10] = Pf[:, 1]   # p01
    src[:, 11] = Y[0, :]
    src[:, 12] = Y[1, :]
    packed = np.zeros((n_cores, rows_per_core, N_IN), np.float32)
    for k in range(n_cores):
        packed[k, :per] = src[k * per:(k + 1) * per]
    # rows-major (RPC, F) -> plane-major (T, 128, F, C)
    pm = packed.reshape(n_cores, T, 128, C, N_IN).transpose(0, 1, 2, 4, 3)
    return np.ascontiguousarray(pm).reshape(n_cores, T, 128, N_IN * C)


def _planes_to_rows(o, F, C=TILE_C, T=TILE_T):
    """(T, 128, F*C) plane-major -> (T*128*C, F) row records."""
    return o.reshape(T, 128, F, C).transpose(0, 1, 3, 2).reshape(-1, F)


def unpack_outputs(outs_d, outs_a, n, per):
    """Per-core plane-major outputs -> (x_new, P_new, K)."""
    d = np.concatenate(
        [_planes_to_rows(ob, N_OD)[:per] for ob in outs_d], axis=0)
    a_ = np.concatenate(
        [_planes_to_rows(ob, N_OA)[:per] for ob in outs_a], axis=0)
    assert d.shape[0] == n and a_.shape[0] == n
    x_new = np.empty((n, 5), np.float32)
    x_new[:, 0] = d[:, 2]   # px
    x_new[:, 1] = d[:, 3]   # py
    x_new[:, 2] = d[:, 1]   # ang
    x_new[:, 3] = d[:, 4]   # xn3
    x_new[:, 4] = d[:, 5]   # xn4
    P_new = np.zeros((n, 25), np.float32)
    P_new[:, 0] = a_[:, 2]                   # [0,0] B00
    P_new[:, 1] = P_new[:, 5] = d[:, 6]      # [0,1],[1,0] B01
    P_new[:, 2] = P_new[:, 10] = d[:, 7]     # [0,2],[2,0] B02
    P_new[:, 6] = a_[:, 3]                   # [1,1] B11
    P_new[:, 7] = P_new[:, 11] = d[:, 8]     # [1,2],[2,1] B12
    P_new[:, 12] = a_[:, 4]                  # [2,2] B22
    P_new[:, 18] = a_[:, 5]                  # [3,3]
    P_new[:, 24] = a_[:, 6]                  # [4,4]
    P_new = P_new.reshape(n, 5, 5)
    K = np.zeros((n, 10), np.float32)
    K[:, 6] = a_[:, 0]                       # [3,0] k3
    K[:, 9] = a_[:, 1]                       # [4,1] k4
    K = K.reshape(n, 5, 2)
    return x_new, P_new, K


_CACHE = {}
_TRACE = False        # test harness can set kernel._TRACE = True
_LAST_RESULT = None   # BassKernelResults of the most recent run


def _get_program(params):
    key = tuple(sorted((k, tuple(v) if isinstance(v, list) else v)
                       for k, v in params.items()))
    if key not in _CACHE:
        _CACHE[key] = build_program(params)
    return _CACHE[key]


def kernel(x, P, a, Y, n1, n2_sigma, n2_kappa, gains, obs_gains):
    from concourse.bass_utils import run_bass_kernel_spmd

    x = np.asarray(x, np.float32)
    P = np.asarray(P, np.float32)
    a = np.asarray(a, np.float32)
    Y = np.asarray(Y, np.float32)
    n = x.shape[0]
    assert n == N_TOTAL and n % N_CORES == 0

    params = _params_from_inputs(n1, n2_sigma, n2_kappa, gains, obs_gains)
    nc = _get_program(params)

    packed = pack_inputs(x, P, a, Y)
    in_maps = [{"inp": np.ascontiguousarray(packed[k])} for k in range(N_CORES)]
    res = run_bass_kernel_spmd(nc, in_maps, core_ids=list(range(N_CORES)),
                               trace=_TRACE)
    global _LAST_RESULT
    _LAST_RESULT = res
    outs_d = [r["out_d"] for r in res.results]
    outs_a = [r["out_a"] for r in res.results]
    return unpack_outputs(outs_d, outs_a, n, n // N_CORES)


# revision 26
# speedup vs baseline: 1.2349x; 1.0260x over previous
"""Trainium2 Bass kernel for nn_BatchBeliefStep (batched EKF step).

Exploits structure of the reference: the Jacobian A has zero rows/cols 3,4,
so P_new is block-diagonal (3x3 block + two diagonal entries), K has only
2 nonzero entries per row, and the innovation covariance S is diagonal.

Per row only 13 input floats and 15 output floats matter; the host packs
these into dense plane-major records (each feature a contiguous (128, C)
block per tile), the device runs a pure elementwise kernel over them
(data-parallel across 8 cores), and the host scatters the results back
into the full-shape outputs.
"""

import math

import numpy as np

N_TOTAL = 500_000
N_CORES = 8
TILE_C = 246          # records per partition per tile (even: DVE 2x mode)
TILE_T = 2            # tiles per core
ROWS_PER_CORE = 128 * TILE_C * TILE_T  # 62976 (62500 real + 476 pad)

N_IN = 13             # packed input features per row
N_OD = 9              # DVE-written output features per row
N_OA = 7              # ACT/GPSIMD-written output features per row

_PI = float(np.float32(math.pi))


def _params_from_inputs(n1, n2_sigma, n2_kappa, gains, obs_gains):
    f = lambda v: float(np.float32(v))
    q = [f(np.exp(np.float32(2.0) * np.float32(v))) for v in np.asarray(n1)]
    e2s = [f(np.exp(np.float32(2.0) * np.float32(v))) for v in np.asarray(n2_sigma)]
    ekh = [f(np.exp(np.float32(v))) for v in np.asarray(n2_kappa)]
    g = [f(v) for v in np.asarray(gains)]
    og = [f(v) for v in np.asarray(obs_gains)]
    return dict(q=q, e2s=e2s, ekh=ekh, g=g, og=og, dt=0.1, eps=1e-6)


def build_program(params, C=TILE_C, T=TILE_T, split_waits=True):
    """Build the Bass program (plane-major layout).

    inp   (T, 128, 13*C): features [x2, a0, a1, x0, x1, p00, p11, p02,
                          p12, p22, p01, y0, y1], each a length-C segment
    out_d (T, 128, 9*C):  [angc*, ang, px, py, xn3, xn4, B01, B02, B12]
    out_a (T, 128, 7*C):  [k3, k4, B00, B11, B22, pn33, pn44]
    (* = dummy; cos-angle slot is only the Sin input)
    """
    import concourse.bass as bass
    import concourse.mybir as mybir
    from concourse.tile import TileContext

    F32 = mybir.dt.float32
    OP = mybir.AluOpType
    ACTF = mybir.ActivationFunctionType

    dt = params["dt"]
    eps = params["eps"]
    g0, g1 = params["g"]
    og0, og1 = params["og"]
    q0, q1, q2, q3, q4 = params["q"]
    e2s0, e2s1 = params["e2s"]
    ekh0, ekh1 = params["ekh"]
    # s00 = vel^2*e2k0 + (e2s0 + og0^2*q3);  k3 = og0*q3 / s00
    f32 = lambda v: float(np.float32(v))
    c0 = f32(np.float32(e2s0) + np.float32(og0) * np.float32(og0) * np.float32(q3))
    c1 = f32(np.float32(e2s1) + np.float32(og1) * np.float32(og1) * np.float32(q4))
    ogq0 = f32(np.float32(og0) * np.float32(q3))
    ogq1 = f32(np.float32(og1) * np.float32(q4))
    qe0, qe1, qe2 = f32(q0 + eps), f32(q1 + eps), f32(q2 + eps)
    qe3, qe4 = f32(q3 + eps), f32(q4 + eps)
    half_pi = f32(_PI / 2)
    MAGIC = 12582912.0  # 1.5 * 2**23: fp32 round-to-nearest-integer trick
    inv2pi = f32(1.0 / (2 * math.pi))
    two_pi = f32(2 * math.pi)

    nc = bass.Bass()
    inp = nc.dram_tensor("inp", (T, 128, N_IN * C), F32, kind="ExternalInput")
    out_d = nc.dram_tensor("out_d", (T, 128, N_OD * C), F32,
                           kind="ExternalOutput")
    out_a = nc.dram_tensor("out_a", (T, 128, N_OA * C), F32,
                           kind="ExternalOutput")

    with TileContext(nc) as tc:
        with tc.tile_pool(name="io", bufs=2) as io, \
             tc.tile_pool(name="scr", bufs=3) as scr:

            for t in range(T):
                it = io.tile([128, N_IN, C], F32, tag="it")
                nc.sync.dma_start(it[:], inp[t])
                x2, a0, a1 = it[:, 0, :], it[:, 1, :], it[:, 2, :]
                x0, x1 = it[:, 3, :], it[:, 4, :]
                p00, p11 = it[:, 5, :], it[:, 6, :]
                p02, p12, p22 = it[:, 7, :], it[:, 8, :], it[:, 9, :]
                p01 = it[:, 10, :]
                y0, y1 = it[:, 11, :], it[:, 12, :]

                od = io.tile([128, N_OD, C], F32, tag="od")
                oANGp = od[:, 0:2, :]          # (angc, ang)
                oPXY = od[:, 2:4, :]
                oXN = od[:, 4:6, :]
                oB01, oB02, oB12 = od[:, 6, :], od[:, 7, :], od[:, 8, :]

                oa = io.tile([128, N_OA, C], F32, tag="oa")
                oK = oa[:, 0:2, :]
                oB00, oB11, oB22 = oa[:, 2, :], oa[:, 3, :], oa[:, 4, :]
                oPN33, oPN44 = oa[:, 5, :], oa[:, 6, :]

                # scratch (feature-slot layout, all unit-stride views)
                SA = scr.tile([128, 8, C], F32, tag="SA")
                PRE = SA[:, 0:2, :]            # (angc_pre, ang_pre)
                KF = SA[:, 2:4, :]
                CS = SA[:, 4:6, :]             # (cos, sin)
                VEL = SA[:, 6:8, :]            # (vel, angvel)
                SB = scr.tile([128, 8, C], F32, tag="SB")
                JJ = SB[:, 0:2, :]             # (j0, j1)
                PP = SB[:, 2:4, :]
                M12 = SB[:, 4:6, :]
                E12 = SB[:, 6:8, :]
                SC = scr.tile([128, 8, C], F32, tag="SC")
                F12 = SC[:, 0:2, :]
                G12 = SC[:, 2:4, :]
                A1, A2, A3 = SC[:, 4, :], SC[:, 5, :], SC[:, 6, :]
                PRD, IP = SC[:, 7, :], SB[:, 4, :]   # IP reuses M12[0] slot
                SD = scr.tile([128, 6, C], F32, tag="SD")
                SQ = SD[:, 0:2, :]
                ER = SD[:, 2:4, :]
                T2 = SD[:, 4:6, :]

                # --- angle: ang = wrap(x2 + angvel*dt); angc = wrap(+pi/2)
                # wrap(y) = y - 2pi*rint(y/2pi); rint via the MAGIC trick
                nc.vector.scalar_tensor_tensor(
                    PRE[:, 1, :], a1, g1 * dt, x2, OP.mult, OP.add)
                nc.vector.tensor_scalar_add(PRE[:, 0, :], PRE[:, 1, :],
                                            half_pi)
                nc.vector.tensor_scalar(KF, PRE, inv2pi, MAGIC,
                                        OP.mult, OP.add)
                nc.vector.tensor_scalar(KF, KF, MAGIC, None, OP.subtract)
                nc.vector.scalar_tensor_tensor(oANGp, KF, -two_pi, PRE,
                                               OP.mult, OP.add)
                # guard against half-ulp overshoot past +/-pi (Sin range)
                nc.vector.tensor_scalar(oANGp, oANGp, -_PI, _PI,
                                        OP.max, OP.min)
                nc.scalar.activation(CS, oANGp, ACTF.Sin)

                # --- vel/angvel (ACT)
                nc.scalar.mul(VEL[:, 0, :], a0, g0)
                nc.scalar.mul(VEL[:, 1, :], a1, g1)

                # --- j0 = -vel*sin*dt; j1 = vel*cos*dt
                nc.vector.scalar_tensor_tensor(
                    JJ[:, 0, :], CS[:, 1, :], -dt, VEL[:, 0, :],
                    OP.mult, OP.mult)
                nc.vector.scalar_tensor_tensor(
                    JJ[:, 1, :], CS[:, 0, :], dt, VEL[:, 0, :],
                    OP.mult, OP.mult)

                # --- px = clip(x0 + j1), py = clip(x1 - j0)
                nc.vector.tensor_add(PP[:, 0, :], JJ[:, 1, :], x0)
                nc.vector.tensor_sub(PP[:, 1, :], x1, JJ[:, 0, :])
                nc.vector.tensor_scalar(oPXY, PP, -1.0, 1.0,
                                        OP.max, OP.min)

                # --- 3x3 covariance block, channels paired over (j0, j1)
                P0212 = it[:, 7:9, :]
                P0011 = it[:, 5:7, :]
                p22b = it[:, 9:10, :].to_broadcast((128, 2, C))
                nc.vector.tensor_mul(M12, JJ, p22b)          # (j0, j1)*p22
                nc.vector.tensor_add(od[:, 7:9, :], M12, P0212)   # B02, B12
                nc.scalar.activation(oB22, p22, ACTF.Copy, bias=qe2)

                nc.vector.tensor_add(E12, od[:, 7:9, :], P0212)
                nc.vector.tensor_mul(F12, JJ, E12)
                nc.vector.tensor_add(G12, P0011, F12)
                nc.scalar.activation(oB00, G12[:, 0, :], ACTF.Copy, bias=qe0)
                nc.scalar.activation(oB11, G12[:, 1, :], ACTF.Copy, bias=qe1)

                nc.vector.tensor_mul(A1, JJ[:, 0, :], oB12)  # j0*B12
                nc.vector.tensor_mul(A2, JJ[:, 1, :], p02)   # j1*p02
                nc.vector.tensor_add(A3, A2, A1)
                nc.vector.tensor_add(oB01, A3, p01)

                # --- Kalman channels: s00, s11; k = og*q*s_other/(s00*s11)
                nc.scalar.activation(SQ[:, 0, :], VEL[:, 0, :], ACTF.Square,
                                     scale=ekh0)
                nc.scalar.activation(SQ[:, 1, :], VEL[:, 1, :], ACTF.Square,
                                     scale=ekh1)
                nc.scalar.activation(SQ[:, 0, :], SQ[:, 0, :], ACTF.Copy,
                                     bias=c0)
                nc.scalar.activation(SQ[:, 1, :], SQ[:, 1, :], ACTF.Copy,
                                     bias=c1)
                nc.vector.tensor_mul(PRD, SQ[:, 0, :], SQ[:, 1, :])
                nc.vector.reciprocal(IP, PRD)
                nc.vector.scalar_tensor_tensor(
                    oa[:, 0, :], SQ[:, 1, :], ogq0, IP, OP.mult, OP.mult)
                nc.vector.scalar_tensor_tensor(
                    oa[:, 1, :], SQ[:, 0, :], ogq1, IP, OP.mult, OP.mult)

                nc.vector.scalar_tensor_tensor(
                    ER[:, 0, :], a0, -(og0 * g0), y0, OP.mult, OP.add)
                nc.vector.scalar_tensor_tensor(
                    ER[:, 1, :], a1, -(og1 * g1), y1, OP.mult, OP.add)
                nc.vector.tensor_mul(T2, oK, ER)
                nc.vector.tensor_add(oXN, T2, VEL)

                nc.scalar.activation(oPN33, oa[:, 0, :], ACTF.Copy,
                                     bias=qe3, scale=-ogq0)
                nc.scalar.activation(oPN44, oa[:, 1, :], ACTF.Copy,
                                     bias=qe4, scale=-ogq1)

                nc.sync.dma_start(out_d[t], od[:])
                nc.sync.dma_start(out_a[t], oa[:])

    if split_waits:
        _split_excess_waits(nc)
    return nc


def _split_excess_waits(nc, keep=1):
    """Walrus's ISA structs here afford very few sem-wait slots per
    instruction. Hoist all but `keep` waits of any instruction onto
    same-engine NOPs inserted right before it (same queue => program order
    guarantees the waits still gate the instruction)."""
    import concourse.mybir as mybir

    ctr = 0
    for f in nc.m.functions:
        for b in f.blocks:
            new_insts = []
            for inst in b.instructions:
                si = getattr(inst, "sync_info", None)
                waits = list(si.on_wait) if (si is not None and si.on_wait) else []
                if len(waits) > keep:
                    excess, kept = waits[:-keep], waits[-keep:]
                    for w in excess:
                        nop = mybir.InstNoOp(name=f"waitnop-{ctr}", ins=[],
                                             outs=[])
                        ctr += 1
                        nop.engine = inst.engine
                        nop.sync_info = mybir.SyncInfo(on_update=[],
                                                       on_wait=[w])
                        new_insts.append(nop)
                    si.on_wait = kept
                new_insts.append(inst)
            b.instructions[:] = new_insts


_IN_PERM = None  # feature order in packed input


def pack_inputs(x, P, a, Y, rows_per_core=ROWS_PER_CORE, n_cores=N_CORES,
                C=TILE_C, T=TILE_T):
    """Pack full inputs into per-core (T, 128, 13*C) plane-major arrays."""
    n = x.shape[0]
    per = n // n_cores
    src = np.empty((n, N_IN), np.float32)
    src[:, 0] = x[:, 2]
    src[:, 1] = a[:, 0]
    src[:, 2] = a[:, 1]
    src[:, 3] = x[:, 0]
    src[:, 4] = x[:, 1]
    Pf = P.reshape(n, 25)
    src[:, 5] = Pf[:, 0]    # p00
    src[:, 6] = Pf[:, 6]    # p11
    src[:, 7] = Pf[:, 2]    # p02
    src[:, 8] = Pf[:, 7]    # p12
    src[:, 9] = Pf[:, 12]   # p22
    src[:, 10] = Pf[:, 1]   # p01
    src[:, 11] = Y[0, :]
    src[:, 12] = Y[1, :]
    packed = np.zeros((n_cores, rows_per_core, N_IN), np.float32)
    for k in range(n_cores):
        packed[k, :per] = src[k * per:(k + 1) * per]
    # rows-major (RPC, F) -> plane-major (T, 128, F, C)
    pm = packed.reshape(n_cores, T, 128, C, N_IN).transpose(0, 1, 2, 4, 3)
    return np.ascontiguousarray(pm).reshape(n_cores, T, 128, N_IN * C)


def _planes_to_rows(o, F, C=TILE_C, T=TILE_T):
    """(T, 128, F*C) plane-major -> (T*128*C, F) row records."""
    return o.reshape(T, 128, F, C).transpose(0, 1, 3, 2).reshape(-1, F)


def unpack_outputs(outs_d, outs_a, n, per):
    """Per-core plane-major outputs -> (x_new, P_new, K)."""
    d = np.concatenate(
        [_planes_to_rows(ob, N_OD)[:per] for ob in outs_d], axis=0)
    a_ = np.concatenate(
        [_planes_to_rows(ob, N_OA)[:per] for ob in outs_a], axis=0)
    assert d.shape[0] == n and a_.shape[0] == n
    x_new = np.empty((n, 5), np.float32)
    x_new[:, 0] = d[:, 2]   # px
    x_new[:, 1] = d[:, 3]   # py
    x_new[:, 2] = d[:, 1]   # ang
    x_new[:, 3] = d[:, 4]   # xn3
    x_new[:, 4] = d[:, 5]   # xn4
    P_new = np.zeros((n, 25), np.float32)
    P_new[:, 0] = a_[:, 2]                   # [0,0] B00
    P_new[:, 1] = P_new[:, 5] = d[:, 6]      # [0,1],[1,0] B01
    P_new[:, 2] = P_new[:, 10] = d[:, 7]     # [0,2],[2,0] B02
    P_new[:, 6] = a_[:, 3]                   # [1,1] B11
    P_new[:, 7] = P_new[:, 11] = d[:, 8]     # [1,2],[2,1] B12
    P_new[:, 12] = a_[:, 4]                  # [2,2] B22
    P_new[:, 18] = a_[:, 5]                  # [3,3]
    P_new[:, 24] = a_[:, 6]                  # [4,4]
    P_new = P_new.reshape(n, 5, 5)
    K = np.zeros((n, 10), np.float32)
    K[:, 6] = a_[:, 0]                       # [3,0] k3
    K[:, 9] = a_[:, 1]                       # [4,1] k4
    K = K.reshape(n, 5, 2)
    return x_new, P_new, K


_CACHE = {}
_TRACE = False        # test harness can set kernel._TRACE = True
_LAST_RESULT = None   # BassKernelResults of the most recent run


def _get_program(params):
    key = tuple(sorted((k, tuple(v) if isinstance(v, list) else v)
                       for k, v in params.items()))
    if key not in _CACHE:
        _CACHE[key] = build_program(params)
    return _CACHE[key]


def kernel(x, P, a, Y, n1, n2_sigma, n2_kappa, gains, obs_gains):
    from concourse.bass_utils import run_bass_kernel_spmd

    x = np.asarray(x, np.float32)
    P = np.asarray(P, np.float32)
    a = np.asarray(a, np.float32)
    Y = np.asarray(Y, np.float32)
    n = x.shape[0]
    assert n == N_TOTAL and n % N_CORES == 0

    params = _params_from_inputs(n1, n2_sigma, n2_kappa, gains, obs_gains)
    nc = _get_program(params)

    packed = pack_inputs(x, P, a, Y)
    in_maps = [{"inp": np.ascontiguousarray(packed[k])} for k in range(N_CORES)]
    res = run_bass_kernel_spmd(nc, in_maps, core_ids=list(range(N_CORES)),
                               trace=_TRACE)
    global _LAST_RESULT
    _LAST_RESULT = res
    outs_d = [r["out_d"] for r in res.results]
    outs_a = [r["out_a"] for r in res.results]
    return unpack_outputs(outs_d, outs_a, n, n // N_CORES)
